# revision 5
# baseline (speedup 1.0000x reference)
"""Routed quantized MoE eval kernel for 8 Trainium2 NeuronCores.

Strategy (expert-parallel, per sharding hint):
- Core c owns expert e=c: quantized expert weights are dequantized
  (scale-folded) + transposed on the host at shard-prep time; the
  matmuls, router, top-2 softmax, SwiGLU activations and combine all
  run on device.
- Shared MLP is sharded along DF_S: core c computes rows
  [256c, 256c+256) of the shared gate/up and the matching columns of
  the down projection, giving a partial shared output.
- Every core computes the full router (fp32 matmuls - top-2 selection
  is tie-sensitive), forms its own expert's combine column
  ca[:, e] * alpha[e] and (1 - sum_e ca*alpha), scales its expert
  output and shared partial, and the per-token sum across all 8 cores
  is taken by chunked ReduceScatter collectives that overlap compute.
- Big matmuls run as float32r (2 cyc/row, ~1.5e-4 rel err), router in
  true float32.

Output identity used:
  mixed = (1 - sum_e ca_e*alpha_e) * shared + sum_e ca_e*alpha_e * eo_e
where shared = sum over cores of shared partials, so each core's
contribution is (1-s)*shared_partial_c + ca_c*alpha_c*eo_c.
"""

import numpy as np
from contextlib import ExitStack

import concourse.bass as bass
import concourse.tile as tile
from concourse import bacc, mybir
from concourse.bass_utils import run_bass_kernel_spmd

NCORES = 8
B, S, D = 2, 1024, 1024
T = B * S                      # 2048 tokens
DF_E, DF_S, E = 512, 2048, 8
FS = DF_S // NCORES            # 256 shared-ffn rows per core
CH = 4                         # token chunks
CT = T // CH                   # 512 tokens per chunk
TT = CT // 128                 # 4 token tiles per chunk
KD = D // 128                  # 8 k-tiles over hidden dim
KF = DF_E // 128               # 4 k-tiles over expert ffn dim
KS = FS // 128                 # 2 k-tiles over shared ffn shard
ND = D // 512                  # 2 output column slices

FR = mybir.dt.float32r
F32 = mybir.dt.float32
ACTF = mybir.ActivationFunctionType
ALU = mybir.AluOpType

_CACHE = {}


def _build():
    nc = bacc.Bacc(
        "TRN2", target_bir_lowering=False, debug=False, num_devices=NCORES
    )

    xT = nc.dram_tensor("xT", [D, T], FR, kind="ExternalInput").ap()
    rwT = nc.dram_tensor("rwT", [D, E], F32, kind="ExternalInput").ap()
    gqT = nc.dram_tensor("gqT", [D, DF_E], FR, kind="ExternalInput").ap()
    uqT = nc.dram_tensor("uqT", [D, DF_E], FR, kind="ExternalInput").ap()
    dqT = nc.dram_tensor("dqT", [DF_E, D], FR, kind="ExternalInput").ap()
    wgT = nc.dram_tensor("wgT", [D, FS], FR, kind="ExternalInput").ap()
    wuT = nc.dram_tensor("wuT", [D, FS], FR, kind="ExternalInput").ap()
    wdT = nc.dram_tensor("wdT", [FS, D], FR, kind="ExternalInput").ap()
    # aux[:, 0:8] = alpha broadcast, aux[:, 8:16] = onehot(expert) broadcast
    aux = nc.dram_tensor("aux", [128, 2 * E], F32, kind="ExternalInput").ap()
    OUT = nc.dram_tensor(
        "OUT", [CH, CT // NCORES, D], F32, kind="ExternalOutput"
    ).ap()

    with ExitStack() as ctx:
        tc = ctx.enter_context(tile.TileContext(nc))
        wres = ctx.enter_context(tc.tile_pool(name="wres", bufs=1))
        xs = ctx.enter_context(tc.tile_pool(name="xs", bufs=2))
        hp = ctx.enter_context(tc.tile_pool(name="hp", bufs=2))
        work = ctx.enter_context(tc.tile_pool(name="work", bufs=2))
        rt = ctx.enter_context(tc.tile_pool(name="rt", bufs=2))
        ps_gu = ctx.enter_context(tc.tile_pool(name="ps_gu", bufs=2, space="PSUM"))
        ps_dn = ctx.enter_context(tc.tile_pool(name="ps_dn", bufs=2, space="PSUM"))
        ps_sd = ctx.enter_context(tc.tile_pool(name="ps_sd", bufs=1, space="PSUM"))
        ps_r = ctx.enter_context(tc.tile_pool(name="ps_r", bufs=1, space="PSUM"))
        dram = ctx.enter_context(tc.tile_pool(name="dram", bufs=1, space="DRAM"))

        # ---- resident weights ----------------------------------------
        def load_rows(src, rows, cols, name):
            tiles = []
            r = src.rearrange("(k p) n -> k p n", p=128)
            for k in range(rows // 128):
                t = wres.tile([128, cols], src.dtype, tag=f"{name}{k}")
                nc.sync.dma_start(t[:], r[k])
                tiles.append(t)
            return tiles

        gq = load_rows(gqT, D, DF_E, "gq")
        uq = load_rows(uqT, D, DF_E, "uq")
        dq = load_rows(dqT, DF_E, D, "dq")
        wg = load_rows(wgT, D, FS, "wg")
        wu = load_rows(wuT, D, FS, "wu")
        wd = load_rows(wdT, FS, D, "wd")
        rw = load_rows(rwT, D, E, "rw")
        aux_sb = wres.tile([128, 2 * E], F32, tag="aux")
        nc.sync.dma_start(aux_sb[:], aux[:])
        alpha_bc = aux_sb[:, 0:E]
        sel_bc = aux_sb[:, E : 2 * E]

        xTr = xT.rearrange("(k p) t -> k p t", p=128)

        for c in range(CH):
            # ---- load x chunk (d-major, tokens on free axis) ---------
            xt = []
            for k in range(KD):
                t = xs.tile([128, CT], FR, tag=f"xt{k}")
                nc.sync.dma_start(t[:], xTr[k, :, c * CT : (c + 1) * CT])
                xt.append(t)

            # ---- router + combine weights ----------------------------
            ca_cols = []
            oneminus = []
            for j in range(TT):
                ps_l = ps_r.tile([128, E], F32, tag="psl")
                for k in range(KD):
                    nc.tensor.matmul(
                        ps_l[:],
                        xt[k][:, j * 128 : (j + 1) * 128].bitcast(F32),
                        rw[k][:],
                        start=(k == 0),
                        stop=(k == KD - 1),
                    )
                L = rt.tile([128, E], F32, tag="L")
                nc.scalar.activation(L[:], ps_l[:], ACTF.Copy)
                m1 = rt.tile([128, 1], F32, tag="m1")
                nc.vector.tensor_reduce(m1[:], L[:], mybir.AxisListType.X, ALU.max)
                mask1 = rt.tile([128, E], F32, tag="mask1")
                nc.vector.tensor_scalar(mask1[:], L[:], m1[:], None, ALU.is_ge)
                L2 = rt.tile([128, E], F32, tag="L2")
                nc.vector.scalar_tensor_tensor(
                    L2[:], mask1[:], -1e30, L[:], ALU.mult, ALU.add
                )
                m2 = rt.tile([128, 1], F32, tag="m2")
                nc.vector.tensor_reduce(m2[:], L2[:], mybir.AxisListType.X, ALU.max)
                mask2 = rt.tile([128, E], F32, tag="mask2")
                nc.vector.tensor_scalar(mask2[:], L2[:], m2[:], None, ALU.is_ge)
                negm1 = rt.tile([128, 1], F32, tag="negm1")
                nc.vector.tensor_scalar_mul(negm1[:], m1[:], -1.0)
                e2 = rt.tile([128, 1], F32, tag="e2")
                nc.scalar.activation(e2[:], m2[:], ACTF.Exp, bias=negm1[:])
                den = rt.tile([128, 1], F32, tag="den")
                nc.vector.tensor_scalar_add(den[:], e2[:], 1.0)
                w1 = rt.tile([128, 1], F32, tag="w1")
                nc.vector.reciprocal(w1[:], den[:])
                w2 = rt.tile([128, 1], F32, tag="w2")
                nc.vector.tensor_mul(w2[:], e2[:], w1[:])
                caw = rt.tile([128, E], F32, tag="caw")
                nc.vector.tensor_scalar(caw[:], mask2[:], w2[:], None, ALU.mult)
                nc.vector.scalar_tensor_tensor(
                    caw[:], mask1[:], w1[:], caw[:], ALU.mult, ALU.add
                )
                ca_a = rt.tile([128, E], F32, tag="ca_a")
                nc.vector.tensor_mul(ca_a[:], caw[:], alpha_bc)
                s = rt.tile([128, 1], F32, tag="s")
                nc.vector.tensor_reduce(s[:], ca_a[:], mybir.AxisListType.X, ALU.add)
                om = rt.tile([128, 1], F32, tag=f"om{j}")
                nc.vector.tensor_scalar(om[:], s[:], -1.0, 1.0, ALU.mult, ALU.add)
                sel = rt.tile([128, E], F32, tag="selm")
                nc.vector.tensor_mul(sel[:], ca_a[:], sel_bc)
                cac = rt.tile([128, 1], F32, tag=f"cac{j}")
                nc.vector.tensor_reduce(
                    cac[:], sel[:], mybir.AxisListType.X, ALU.add
                )
                ca_cols.append(cac)
                oneminus.append(om)

            # ---- expert gate/up + SwiGLU -> h [DF_E, CT] -------------
            hc = []
            for f in range(KF):
                psg = ps_gu.tile([128, CT], F32, tag="psg")
                for k in range(KD):
                    nc.tensor.matmul(
                        psg[:],
                        gq[k][:, f * 128 : (f + 1) * 128],
                        xt[k][:],
                        start=(k == 0),
                        stop=(k == KD - 1),
                    )
                psu = ps_gu.tile([128, CT], F32, tag="psu")
                for k in range(KD):
                    nc.tensor.matmul(
                        psu[:],
                        uq[k][:, f * 128 : (f + 1) * 128],
                        xt[k][:],
                        start=(k == 0),
                        stop=(k == KD - 1),
                    )
                sig = work.tile([128, CT], F32, tag="sig")
                nc.scalar.activation(sig[:], psg[:], ACTF.Sigmoid)
                sil = work.tile([128, CT], F32, tag="sil")
                nc.vector.tensor_mul(sil[:], sig[:], psg[:])
                h = hp.tile([128, CT], FR, tag=f"h{f}")
                nc.vector.tensor_mul(h[:], sil[:], psu[:])
                hc.append(h)

            # ---- shared gate/up -> hs [FS, CT] -----------------------
            hsc = []
            for f in range(KS):
                psg = ps_gu.tile([128, CT], F32, tag="psg")
                for k in range(KD):
                    nc.tensor.matmul(
                        psg[:],
                        wg[k][:, f * 128 : (f + 1) * 128],
                        xt[k][:],
                        start=(k == 0),
                        stop=(k == KD - 1),
                    )
                psu = ps_gu.tile([128, CT], F32, tag="psu")
                for k in range(KD):
                    nc.tensor.matmul(
                        psu[:],
                        wu[k][:, f * 128 : (f + 1) * 128],
                        xt[k][:],
                        start=(k == 0),
                        stop=(k == KD - 1),
                    )
                sig = work.tile([128, CT], F32, tag="sig")
                nc.scalar.activation(sig[:], psg[:], ACTF.Sigmoid)
                sil = work.tile([128, CT], F32, tag="sil")
                nc.vector.tensor_mul(sil[:], sig[:], psg[:])
                hs = hp.tile([128, CT], FR, tag=f"hs{f}")
                nc.vector.tensor_mul(hs[:], sil[:], psu[:])
                hsc.append(hs)

            # ---- down projections + combine + scatter to bounce ------
            rs_in = dram.tile([CT, D], F32, tag=f"rsin{c}")
            rs_out = dram.tile([CT // NCORES, D], F32, tag=f"rsout{c}")
            for j in range(TT):
                for dd in range(ND):
                    pse = ps_dn.tile([128, 512], F32, tag="pse")
                    for k in range(KF):
                        nc.tensor.matmul(
                            pse[:],
                            hc[k][:, j * 128 : (j + 1) * 128],
                            dq[k][:, dd * 512 : (dd + 1) * 512],
                            start=(k == 0),
                            stop=(k == KF - 1),
                        )
                    pss = ps_sd.tile([128, 512], F32, tag="pss")
                    for k in range(KS):
                        nc.tensor.matmul(
                            pss[:],
                            hsc[k][:, j * 128 : (j + 1) * 128],
                            wd[k][:, dd * 512 : (dd + 1) * 512],
                            start=(k == 0),
                            stop=(k == KS - 1),
                        )
                    esc = work.tile([128, 512], F32, tag="esc")
                    nc.scalar.activation(
                        esc[:], pse[:], ACTF.Copy, scale=ca_cols[j][:]
                    )
                    contrib = work.tile([128, 512], F32, tag="contrib")
                    nc.vector.scalar_tensor_tensor(
                        contrib[:],
                        pss[:],
                        oneminus[j][:],
                        esc[:],
                        ALU.mult,
                        ALU.add,
                    )
                    nc.sync.dma_start(
                        rs_in[j * 128 : (j + 1) * 128, dd * 512 : (dd + 1) * 512],
                        contrib[:],
                    )

            nc.gpsimd.collective_compute(
                "ReduceScatter",
                ALU.add,
                replica_groups=[list(range(NCORES))],
                ins=[rs_in.opt()],
                outs=[rs_out.opt()],
            )
            nc.sync.dma_start(OUT[c], rs_out[:])

    nc.compile()
    return nc


def _prep_inputs(x, router_weight, sh_gate_w, sh_up_w, sh_down_w, gate_s,
                 up_s, down_s, alpha, gate_q, up_q, down_q):
    xf = np.ascontiguousarray(
        np.asarray(x, dtype=np.float32).reshape(T, D).T
    )
    rwT = np.ascontiguousarray(np.asarray(router_weight, np.float32).T)
    in_maps = []
    for c in range(NCORES):
        gw = np.asarray(gate_q[c], np.float32) * np.asarray(
            gate_s[c], np.float32
        )[:, None]                                  # [DF_E, D]
        uw = np.asarray(up_q[c], np.float32) * np.asarray(
            up_s[c], np.float32
        )[:, None]                                  # [DF_E, D]
        dw = np.asarray(down_q[c], np.float32) * np.asarray(
            down_s[c], np.float32
        )[:, None]                                  # [D, DF_E]
        aux = np.zeros((128, 2 * E), np.float32)
        aux[:, 0:E] = np.asarray(alpha, np.float32)[None, :]
        aux[:, E + c] = 1.0
        in_maps.append(
            {
                "xT": xf,
                "rwT": rwT,
                "gqT": np.ascontiguousarray(gw.T),
                "uqT": np.ascontiguousarray(uw.T),
                "dqT": np.ascontiguousarray(dw.T),
                "wgT": np.ascontiguousarray(
                    np.asarray(sh_gate_w[c * FS : (c + 1) * FS], np.float32).T
                ),
                "wuT": np.ascontiguousarray(
                    np.asarray(sh_up_w[c * FS : (c + 1) * FS], np.float32).T
                ),
                "wdT": np.ascontiguousarray(
                    np.asarray(sh_down_w[:, c * FS : (c + 1) * FS], np.float32).T
                ),
                "aux": aux,
            }
        )
    return in_maps


def kernel(x, router_weight, sh_gate_w, sh_up_w, sh_down_w, gate_s, up_s,
           down_s, alpha, gate_q, up_q, down_q, top_k, **run_kwargs):
    assert int(top_k) == 2, "kernel compiled for top_k=2"
    assert tuple(np.shape(x)) == (B, S, D)

    if "nc" not in _CACHE:
        _CACHE["nc"] = _build()
    nc = _CACHE["nc"]

    in_maps = _prep_inputs(
        x, router_weight, sh_gate_w, sh_up_w, sh_down_w, gate_s, up_s,
        down_s, alpha, gate_q, up_q, down_q,
    )
    res = run_bass_kernel_spmd(
        nc, in_maps, core_ids=list(range(NCORES)), **run_kwargs
    )
    _CACHE["last_results"] = res

    out = np.empty((T, D), np.float32)
    sh = CT // NCORES  # 64 rows per (chunk, rank)
    for r in range(NCORES):
        o = res.results[r]["OUT"]  # [CH, 64, D]
        for c in range(CH):
            out[c * CT + r * sh : c * CT + (r + 1) * sh] = o[c]
    return out.reshape(B, S, D).astype(np.asarray(x).dtype)


# revision 6
# speedup vs baseline: 1.0121x; 1.0121x over previous
"""Routed quantized MoE eval kernel for 8 Trainium2 NeuronCores.

Strategy (expert-parallel, per sharding hint):
- Core c owns expert e=c: quantized expert weights are dequantized
  (scale-folded) + transposed on the host at shard-prep time; the
  matmuls, router, top-2 softmax, SwiGLU activations and combine all
  run on device.
- Shared MLP is sharded along DF_S: core c computes rows
  [256c, 256c+256) of the shared gate/up and the matching columns of
  the down projection, giving a partial shared output.
- Every core computes the full router (fp32 matmuls - top-2 selection
  is tie-sensitive), forms its own expert's combine column
  ca[:, e] * alpha[e] and (1 - sum_e ca*alpha), scales its expert
  output and shared partial, and the per-token sum across all 8 cores
  is taken by chunked ReduceScatter collectives that overlap compute.
- Big matmuls run as float32r (2 cyc/row, ~1.5e-4 rel err), router in
  true float32.

Output identity used:
  mixed = (1 - sum_e ca_e*alpha_e) * shared + sum_e ca_e*alpha_e * eo_e
where shared = sum over cores of shared partials, so each core's
contribution is (1-s)*shared_partial_c + ca_c*alpha_c*eo_c.
"""

import numpy as np
from contextlib import ExitStack

import concourse.bass as bass
import concourse.tile as tile
from concourse import bacc, mybir
from concourse.bass_utils import run_bass_kernel_spmd

NCORES = 8
B, S, D = 2, 1024, 1024
T = B * S                      # 2048 tokens
DF_E, DF_S, E = 512, 2048, 8
FS = DF_S // NCORES            # 256 shared-ffn rows per core
CH = 4                         # token chunks
CT = T // CH                   # 512 tokens per chunk
TT = CT // 128                 # 4 token tiles per chunk
KD = D // 128                  # 8 k-tiles over hidden dim
KF = DF_E // 128               # 4 k-tiles over expert ffn dim
KS = FS // 128                 # 2 k-tiles over shared ffn shard
ND = D // 512                  # 2 output column slices

import os

FR = mybir.dt.float32r
F16 = mybir.dt.float16
F32 = mybir.dt.float32
DT_MODE = os.environ.get("MOE_DT", "f32r")
DT_MM = {"f32r": FR, "f16": F16}[DT_MODE]
NP_MM = {"f32r": np.float32, "f16": np.float16}[DT_MODE]
ACTF = mybir.ActivationFunctionType
ALU = mybir.AluOpType

_CACHE = {}


def _build():
    nc = bacc.Bacc(
        "TRN2", target_bir_lowering=False, debug=False, num_devices=NCORES
    )

    xT = nc.dram_tensor("xT", [D, T], DT_MM, kind="ExternalInput").ap()
    xTf = nc.dram_tensor("xTf", [D, T], F32, kind="ExternalInput").ap()
    rwT = nc.dram_tensor("rwT", [D, E], F32, kind="ExternalInput").ap()
    gqT = nc.dram_tensor("gqT", [D, DF_E], DT_MM, kind="ExternalInput").ap()
    uqT = nc.dram_tensor("uqT", [D, DF_E], DT_MM, kind="ExternalInput").ap()
    dqT = nc.dram_tensor("dqT", [DF_E, D], DT_MM, kind="ExternalInput").ap()
    wgT = nc.dram_tensor("wgT", [D, FS], DT_MM, kind="ExternalInput").ap()
    wuT = nc.dram_tensor("wuT", [D, FS], DT_MM, kind="ExternalInput").ap()
    wdT = nc.dram_tensor("wdT", [FS, D], DT_MM, kind="ExternalInput").ap()
    # aux[:, 0:8] = alpha broadcast, aux[:, 8:16] = onehot(expert) broadcast
    aux = nc.dram_tensor("aux", [128, 2 * E], F32, kind="ExternalInput").ap()
    OUT = nc.dram_tensor(
        "OUT", [CH, CT // NCORES, D], F32, kind="ExternalOutput"
    ).ap()

    with ExitStack() as ctx:
        tc = ctx.enter_context(tile.TileContext(nc))
        wres = ctx.enter_context(tc.tile_pool(name="wres", bufs=1))
        xs = ctx.enter_context(tc.tile_pool(name="xs", bufs=2))
        xfp = ctx.enter_context(tc.tile_pool(name="xfp", bufs=1))
        hp = ctx.enter_context(tc.tile_pool(name="hp", bufs=2))
        work = ctx.enter_context(tc.tile_pool(name="work", bufs=2))
        rt = ctx.enter_context(tc.tile_pool(name="rt", bufs=2))
        ps_gu = ctx.enter_context(tc.tile_pool(name="ps_gu", bufs=2, space="PSUM"))
        ps_dn = ctx.enter_context(tc.tile_pool(name="ps_dn", bufs=2, space="PSUM"))
        ps_sd = ctx.enter_context(tc.tile_pool(name="ps_sd", bufs=1, space="PSUM"))
        ps_r = ctx.enter_context(tc.tile_pool(name="ps_r", bufs=1, space="PSUM"))
        dram = ctx.enter_context(tc.tile_pool(name="dram", bufs=1, space="DRAM"))

        # ---- resident weights ----------------------------------------
        def load_rows(src, rows, cols, name):
            tiles = []
            r = src.rearrange("(k p) n -> k p n", p=128)
            for k in range(rows // 128):
                t = wres.tile([128, cols], src.dtype, tag=f"{name}{k}")
                nc.sync.dma_start(t[:], r[k])
                tiles.append(t)
            return tiles

        gq = load_rows(gqT, D, DF_E, "gq")
        uq = load_rows(uqT, D, DF_E, "uq")
        dq = load_rows(dqT, DF_E, D, "dq")
        wg = load_rows(wgT, D, FS, "wg")
        wu = load_rows(wuT, D, FS, "wu")
        wd = load_rows(wdT, FS, D, "wd")
        rw = load_rows(rwT, D, E, "rw")
        aux_sb = wres.tile([128, 2 * E], F32, tag="aux")
        nc.sync.dma_start(aux_sb[:], aux[:])
        alpha_bc = aux_sb[:, 0:E]
        sel_bc = aux_sb[:, E : 2 * E]

        xTr = xT.rearrange("(k p) t -> k p t", p=128)
        xTfr = xTf.rearrange("(k p) t -> k p t", p=128)

        for c in range(CH):
            # ---- load x chunk (d-major, tokens on free axis) ---------
            xt = []
            xf_t = []
            for k in range(KD):
                t = xs.tile([128, CT], DT_MM, tag=f"xt{k}")
                nc.sync.dma_start(t[:], xTr[k, :, c * CT : (c + 1) * CT])
                xt.append(t)
                tf = xfp.tile([128, CT], F32, tag=f"xf{k}")
                nc.sync.dma_start(tf[:], xTfr[k, :, c * CT : (c + 1) * CT])
                xf_t.append(tf)

            # ---- router + combine weights ----------------------------
            ca_cols = []
            oneminus = []
            for j in range(TT):
                ps_l = ps_r.tile([128, E], F32, tag="psl")
                for k in range(KD):
                    nc.tensor.matmul(
                        ps_l[:],
                        xf_t[k][:, j * 128 : (j + 1) * 128],
                        rw[k][:],
                        start=(k == 0),
                        stop=(k == KD - 1),
                    )
                L = rt.tile([128, E], F32, tag="L")
                nc.scalar.activation(L[:], ps_l[:], ACTF.Copy)
                m1 = rt.tile([128, 1], F32, tag="m1")
                nc.vector.tensor_reduce(m1[:], L[:], mybir.AxisListType.X, ALU.max)
                mask1 = rt.tile([128, E], F32, tag="mask1")
                nc.vector.tensor_scalar(mask1[:], L[:], m1[:], None, ALU.is_ge)
                L2 = rt.tile([128, E], F32, tag="L2")
                nc.vector.scalar_tensor_tensor(
                    L2[:], mask1[:], -1e30, L[:], ALU.mult, ALU.add
                )
                m2 = rt.tile([128, 1], F32, tag="m2")
                nc.vector.tensor_reduce(m2[:], L2[:], mybir.AxisListType.X, ALU.max)
                mask2 = rt.tile([128, E], F32, tag="mask2")
                nc.vector.tensor_scalar(mask2[:], L2[:], m2[:], None, ALU.is_ge)
                negm1 = rt.tile([128, 1], F32, tag="negm1")
                nc.vector.tensor_scalar_mul(negm1[:], m1[:], -1.0)
                e2 = rt.tile([128, 1], F32, tag="e2")
                nc.scalar.activation(e2[:], m2[:], ACTF.Exp, bias=negm1[:])
                den = rt.tile([128, 1], F32, tag="den")
                nc.vector.tensor_scalar_add(den[:], e2[:], 1.0)
                w1 = rt.tile([128, 1], F32, tag="w1")
                nc.vector.reciprocal(w1[:], den[:])
                w2 = rt.tile([128, 1], F32, tag="w2")
                nc.vector.tensor_mul(w2[:], e2[:], w1[:])
                caw = rt.tile([128, E], F32, tag="caw")
                nc.vector.tensor_scalar(caw[:], mask2[:], w2[:], None, ALU.mult)
                nc.vector.scalar_tensor_tensor(
                    caw[:], mask1[:], w1[:], caw[:], ALU.mult, ALU.add
                )
                ca_a = rt.tile([128, E], F32, tag="ca_a")
                nc.vector.tensor_mul(ca_a[:], caw[:], alpha_bc)
                s = rt.tile([128, 1], F32, tag="s")
                nc.vector.tensor_reduce(s[:], ca_a[:], mybir.AxisListType.X, ALU.add)
                om = rt.tile([128, 1], F32, tag=f"om{j}")
                nc.vector.tensor_scalar(om[:], s[:], -1.0, 1.0, ALU.mult, ALU.add)
                sel = rt.tile([128, E], F32, tag="selm")
                nc.vector.tensor_mul(sel[:], ca_a[:], sel_bc)
                cac = rt.tile([128, 1], F32, tag=f"cac{j}")
                nc.vector.tensor_reduce(
                    cac[:], sel[:], mybir.AxisListType.X, ALU.add
                )
                ca_cols.append(cac)
                oneminus.append(om)

            # ---- expert gate/up + SwiGLU -> h [DF_E, CT] -------------
            hc = []
            for f in range(KF):
                psg = ps_gu.tile([128, CT], F32, tag="psg")
                for k in range(KD):
                    nc.tensor.matmul(
                        psg[:],
                        gq[k][:, f * 128 : (f + 1) * 128],
                        xt[k][:],
                        start=(k == 0),
                        stop=(k == KD - 1),
                    )
                psu = ps_gu.tile([128, CT], F32, tag="psu")
                for k in range(KD):
                    nc.tensor.matmul(
                        psu[:],
                        uq[k][:, f * 128 : (f + 1) * 128],
                        xt[k][:],
                        start=(k == 0),
                        stop=(k == KD - 1),
                    )
                sig = work.tile([128, CT], F32, tag="sig")
                nc.scalar.activation(sig[:], psg[:], ACTF.Sigmoid)
                sil = work.tile([128, CT], F32, tag="sil")
                nc.vector.tensor_mul(sil[:], sig[:], psg[:])
                h = hp.tile([128, CT], DT_MM, tag=f"h{f}")
                nc.vector.tensor_mul(h[:], sil[:], psu[:])
                hc.append(h)

            # ---- shared gate/up -> hs [FS, CT] -----------------------
            hsc = []
            for f in range(KS):
                psg = ps_gu.tile([128, CT], F32, tag="psg")
                for k in range(KD):
                    nc.tensor.matmul(
                        psg[:],
                        wg[k][:, f * 128 : (f + 1) * 128],
                        xt[k][:],
                        start=(k == 0),
                        stop=(k == KD - 1),
                    )
                psu = ps_gu.tile([128, CT], F32, tag="psu")
                for k in range(KD):
                    nc.tensor.matmul(
                        psu[:],
                        wu[k][:, f * 128 : (f + 1) * 128],
                        xt[k][:],
                        start=(k == 0),
                        stop=(k == KD - 1),
                    )
                sig = work.tile([128, CT], F32, tag="sig")
                nc.scalar.activation(sig[:], psg[:], ACTF.Sigmoid)
                sil = work.tile([128, CT], F32, tag="sil")
                nc.vector.tensor_mul(sil[:], sig[:], psg[:])
                hs = hp.tile([128, CT], DT_MM, tag=f"hs{f}")
                nc.vector.tensor_mul(hs[:], sil[:], psu[:])
                hsc.append(hs)

            # ---- down projections + combine + scatter to bounce ------
            rs_in = dram.tile([CT, D], F32, tag=f"rsin{c}")
            rs_out = dram.tile([CT // NCORES, D], F32, tag=f"rsout{c}")
            for j in range(TT):
                for dd in range(ND):
                    pse = ps_dn.tile([128, 512], F32, tag="pse")
                    for k in range(KF):
                        nc.tensor.matmul(
                            pse[:],
                            hc[k][:, j * 128 : (j + 1) * 128],
                            dq[k][:, dd * 512 : (dd + 1) * 512],
                            start=(k == 0),
                            stop=(k == KF - 1),
                        )
                    pss = ps_sd.tile([128, 512], F32, tag="pss")
                    for k in range(KS):
                        nc.tensor.matmul(
                            pss[:],
                            hsc[k][:, j * 128 : (j + 1) * 128],
                            wd[k][:, dd * 512 : (dd + 1) * 512],
                            start=(k == 0),
                            stop=(k == KS - 1),
                        )
                    esc = work.tile([128, 512], F32, tag="esc")
                    nc.scalar.activation(
                        esc[:], pse[:], ACTF.Copy, scale=ca_cols[j][:]
                    )
                    contrib = work.tile([128, 512], F32, tag="contrib")
                    nc.vector.scalar_tensor_tensor(
                        contrib[:],
                        pss[:],
                        oneminus[j][:],
                        esc[:],
                        ALU.mult,
                        ALU.add,
                    )
                    nc.sync.dma_start(
                        rs_in[j * 128 : (j + 1) * 128, dd * 512 : (dd + 1) * 512],
                        contrib[:],
                    )

            nc.gpsimd.collective_compute(
                "ReduceScatter",
                ALU.add,
                replica_groups=[list(range(NCORES))],
                ins=[rs_in.opt()],
                outs=[rs_out.opt()],
            )
            nc.sync.dma_start(OUT[c], rs_out[:])

    nc.compile()
    return nc


def _prep_inputs(x, router_weight, sh_gate_w, sh_up_w, sh_down_w, gate_s,
                 up_s, down_s, alpha, gate_q, up_q, down_q):
    xf32 = np.ascontiguousarray(
        np.asarray(x, dtype=np.float32).reshape(T, D).T
    )
    xf = np.ascontiguousarray(xf32.astype(NP_MM))
    rwT = np.ascontiguousarray(np.asarray(router_weight, np.float32).T)
    in_maps = []
    for c in range(NCORES):
        gw = np.asarray(gate_q[c], np.float32) * np.asarray(
            gate_s[c], np.float32
        )[:, None]                                  # [DF_E, D]
        uw = np.asarray(up_q[c], np.float32) * np.asarray(
            up_s[c], np.float32
        )[:, None]                                  # [DF_E, D]
        dw = np.asarray(down_q[c], np.float32) * np.asarray(
            down_s[c], np.float32
        )[:, None]                                  # [D, DF_E]
        aux = np.zeros((128, 2 * E), np.float32)
        aux[:, 0:E] = np.asarray(alpha, np.float32)[None, :]
        aux[:, E + c] = 1.0
        in_maps.append(
            {
                "xT": xf,
                "xTf": xf32,
                "rwT": rwT,
                "gqT": np.ascontiguousarray(gw.T.astype(NP_MM)),
                "uqT": np.ascontiguousarray(uw.T.astype(NP_MM)),
                "dqT": np.ascontiguousarray(dw.T.astype(NP_MM)),
                "wgT": np.ascontiguousarray(
                    np.asarray(sh_gate_w[c * FS : (c + 1) * FS], np.float32)
                    .T.astype(NP_MM)
                ),
                "wuT": np.ascontiguousarray(
                    np.asarray(sh_up_w[c * FS : (c + 1) * FS], np.float32)
                    .T.astype(NP_MM)
                ),
                "wdT": np.ascontiguousarray(
                    np.asarray(sh_down_w[:, c * FS : (c + 1) * FS], np.float32)
                    .T.astype(NP_MM)
                ),
                "aux": aux,
            }
        )
    return in_maps


def kernel(x, router_weight, sh_gate_w, sh_up_w, sh_down_w, gate_s, up_s,
           down_s, alpha, gate_q, up_q, down_q, top_k, **run_kwargs):
    assert int(top_k) == 2, "kernel compiled for top_k=2"
    assert tuple(np.shape(x)) == (B, S, D)

    if "nc" not in _CACHE:
        _CACHE["nc"] = _build()
    nc = _CACHE["nc"]

    in_maps = _prep_inputs(
        x, router_weight, sh_gate_w, sh_up_w, sh_down_w, gate_s, up_s,
        down_s, alpha, gate_q, up_q, down_q,
    )
    res = run_bass_kernel_spmd(
        nc, in_maps, core_ids=list(range(NCORES)), **run_kwargs
    )
    _CACHE["last_results"] = res

    out = np.empty((T, D), np.float32)
    sh = CT // NCORES  # 64 rows per (chunk, rank)
    for r in range(NCORES):
        o = res.results[r]["OUT"]  # [CH, 64, D]
        for c in range(CH):
            out[c * CT + r * sh : c * CT + (r + 1) * sh] = o[c]
    return out.reshape(B, S, D).astype(np.asarray(x).dtype)


# revision 8
# speedup vs baseline: 1.0505x; 1.0379x over previous
"""Routed quantized MoE eval kernel for 8 Trainium2 NeuronCores.

Strategy (expert-parallel, per sharding hint):
- Core c owns expert e=c: quantized expert weights are dequantized
  (scale-folded) + transposed on the host at shard-prep time; the
  matmuls, router, top-2 softmax, SwiGLU activations and combine all
  run on device.
- Shared MLP is sharded along DF_S: core c computes rows
  [256c, 256c+256) of the shared gate/up and the matching columns of
  the down projection, giving a partial shared output.
- Every core computes the full router (fp32 matmuls - top-2 selection
  is tie-sensitive), forms its own expert's combine column
  ca[:, e] * alpha[e] and (1 - sum_e ca*alpha), scales its expert
  output and shared partial, and the per-token sum across all 8 cores
  is taken by chunked ReduceScatter collectives that overlap compute.
- Big matmuls run as float32r (2 cyc/row, ~1.5e-4 rel err), router in
  true float32.

Output identity used:
  mixed = (1 - sum_e ca_e*alpha_e) * shared + sum_e ca_e*alpha_e * eo_e
where shared = sum over cores of shared partials, so each core's
contribution is (1-s)*shared_partial_c + ca_c*alpha_c*eo_c.
"""

import numpy as np
from contextlib import ExitStack

import concourse.bass as bass
import concourse.tile as tile
from concourse import bacc, mybir
from concourse.bass_utils import run_bass_kernel_spmd

NCORES = 8
B, S, D = 2, 1024, 1024
T = B * S                      # 2048 tokens
DF_E, DF_S, E = 512, 2048, 8
FS = DF_S // NCORES            # 256 shared-ffn rows per core
CH = 4                         # token chunks
CT = T // CH                   # 512 tokens per chunk
TT = CT // 128                 # 4 token tiles per chunk
KD = D // 128                  # 8 k-tiles over hidden dim
KF = DF_E // 128               # 4 k-tiles over expert ffn dim
KS = FS // 128                 # 2 k-tiles over shared ffn shard
ND = D // 512                  # 2 output column slices

import os

FR = mybir.dt.float32r
F16 = mybir.dt.float16
F32 = mybir.dt.float32
DT_MODE = os.environ.get("MOE_DT", "f32r")
DT_MM = {"f32r": FR, "f16": F16}[DT_MODE]
NP_MM = {"f32r": np.float32, "f16": np.float16}[DT_MODE]
ACTF = mybir.ActivationFunctionType
ALU = mybir.AluOpType

_CACHE = {}


def _build():
    nc = bacc.Bacc(
        "TRN2", target_bir_lowering=False, debug=False, num_devices=NCORES
    )

    xT = nc.dram_tensor("xT", [D, T], DT_MM, kind="ExternalInput").ap()
    xTf = nc.dram_tensor("xTf", [D, T], F32, kind="ExternalInput").ap()
    rwT = nc.dram_tensor("rwT", [D, E], F32, kind="ExternalInput").ap()
    gqT = nc.dram_tensor("gqT", [D, DF_E], DT_MM, kind="ExternalInput").ap()
    uqT = nc.dram_tensor("uqT", [D, DF_E], DT_MM, kind="ExternalInput").ap()
    dqT = nc.dram_tensor("dqT", [DF_E, D], DT_MM, kind="ExternalInput").ap()
    wgT = nc.dram_tensor("wgT", [D, FS], DT_MM, kind="ExternalInput").ap()
    wuT = nc.dram_tensor("wuT", [D, FS], DT_MM, kind="ExternalInput").ap()
    wdT = nc.dram_tensor("wdT", [FS, D], DT_MM, kind="ExternalInput").ap()
    # aux[:, 0:8] = alpha broadcast, aux[:, 8:16] = onehot(expert) broadcast
    aux = nc.dram_tensor("aux", [128, 2 * E], F32, kind="ExternalInput").ap()
    OUT = nc.dram_tensor(
        "OUT", [CH, CT // NCORES, D], F32, kind="ExternalOutput"
    ).ap()

    with ExitStack() as ctx:
        tc = ctx.enter_context(tile.TileContext(nc))
        wres = ctx.enter_context(tc.tile_pool(name="wres", bufs=1))
        xs = ctx.enter_context(tc.tile_pool(name="xs", bufs=2))
        xfp = ctx.enter_context(tc.tile_pool(name="xfp", bufs=1))
        hp = ctx.enter_context(tc.tile_pool(name="hp", bufs=2))
        work = ctx.enter_context(tc.tile_pool(name="work", bufs=2))
        rt = ctx.enter_context(tc.tile_pool(name="rt", bufs=2))
        ps_gu = ctx.enter_context(tc.tile_pool(name="ps_gu", bufs=2, space="PSUM"))
        ps_dn = ctx.enter_context(tc.tile_pool(name="ps_dn", bufs=2, space="PSUM"))
        ps_sd = ctx.enter_context(tc.tile_pool(name="ps_sd", bufs=1, space="PSUM"))
        ps_r = ctx.enter_context(tc.tile_pool(name="ps_r", bufs=1, space="PSUM"))
        dram = ctx.enter_context(tc.tile_pool(name="dram", bufs=1, space="DRAM"))

        # ---- resident weights ----------------------------------------
        def load_rows(src, rows, cols, name):
            tiles = []
            r = src.rearrange("(k p) n -> k p n", p=128)
            for k in range(rows // 128):
                t = wres.tile([128, cols], src.dtype, tag=f"{name}{k}")
                nc.sync.dma_start(t[:], r[k])
                tiles.append(t)
            return tiles

        # issue in first-use order so the PE can start ASAP
        rw = load_rows(rwT, D, E, "rw")
        aux_sb = wres.tile([128, 2 * E], F32, tag="aux")
        nc.sync.dma_start(aux_sb[:], aux[:])
        gq = load_rows(gqT, D, DF_E, "gq")
        uq = load_rows(uqT, D, DF_E, "uq")
        dq = load_rows(dqT, DF_E, D, "dq")
        wg = load_rows(wgT, D, FS, "wg")
        wu = load_rows(wuT, D, FS, "wu")
        wd = load_rows(wdT, FS, D, "wd")
        alpha_bc = aux_sb[:, 0:E]
        sel_bc = aux_sb[:, E : 2 * E]

        xTr = xT.rearrange("(k p) t -> k p t", p=128)
        xTfr = xTf.rearrange("(k p) t -> k p t", p=128)

        for c in range(CH):
            # ---- load x chunk (d-major, tokens on free axis) ---------
            xt = []
            xf_t = []
            for k in range(KD):
                t = xs.tile([128, CT], DT_MM, tag=f"xt{k}")
                nc.sync.dma_start(t[:], xTr[k, :, c * CT : (c + 1) * CT])
                xt.append(t)
                tf = xfp.tile([128, CT], F32, tag=f"xf{k}")
                nc.sync.dma_start(tf[:], xTfr[k, :, c * CT : (c + 1) * CT])
                xf_t.append(tf)

            # ---- router + combine weights ----------------------------
            ca_cols = []
            oneminus = []
            for j in range(TT):
                ps_l = ps_r.tile([128, E], F32, tag="psl")
                for k in range(KD):
                    nc.tensor.matmul(
                        ps_l[:],
                        xf_t[k][:, j * 128 : (j + 1) * 128],
                        rw[k][:],
                        start=(k == 0),
                        stop=(k == KD - 1),
                    )
                L = rt.tile([128, E], F32, tag="L")
                nc.scalar.activation(L[:], ps_l[:], ACTF.Copy)
                m1 = rt.tile([128, 1], F32, tag="m1")
                nc.vector.tensor_reduce(m1[:], L[:], mybir.AxisListType.X, ALU.max)
                mask1 = rt.tile([128, E], F32, tag="mask1")
                nc.vector.tensor_scalar(mask1[:], L[:], m1[:], None, ALU.is_ge)
                L2 = rt.tile([128, E], F32, tag="L2")
                nc.vector.scalar_tensor_tensor(
                    L2[:], mask1[:], -1e30, L[:], ALU.mult, ALU.add
                )
                m2 = rt.tile([128, 1], F32, tag="m2")
                nc.vector.tensor_reduce(m2[:], L2[:], mybir.AxisListType.X, ALU.max)
                mask2 = rt.tile([128, E], F32, tag="mask2")
                nc.vector.tensor_scalar(mask2[:], L2[:], m2[:], None, ALU.is_ge)
                negm1 = rt.tile([128, 1], F32, tag="negm1")
                nc.vector.tensor_scalar_mul(negm1[:], m1[:], -1.0)
                e2 = rt.tile([128, 1], F32, tag="e2")
                nc.scalar.activation(e2[:], m2[:], ACTF.Exp, bias=negm1[:])
                den = rt.tile([128, 1], F32, tag="den")
                nc.vector.tensor_scalar_add(den[:], e2[:], 1.0)
                w1 = rt.tile([128, 1], F32, tag="w1")
                nc.vector.reciprocal(w1[:], den[:])
                w2 = rt.tile([128, 1], F32, tag="w2")
                nc.vector.tensor_mul(w2[:], e2[:], w1[:])
                caw = rt.tile([128, E], F32, tag="caw")
                nc.vector.tensor_scalar(caw[:], mask2[:], w2[:], None, ALU.mult)
                nc.vector.scalar_tensor_tensor(
                    caw[:], mask1[:], w1[:], caw[:], ALU.mult, ALU.add
                )
                ca_a = rt.tile([128, E], F32, tag="ca_a")
                nc.vector.tensor_mul(ca_a[:], caw[:], alpha_bc)
                s = rt.tile([128, 1], F32, tag="s")
                nc.vector.tensor_reduce(s[:], ca_a[:], mybir.AxisListType.X, ALU.add)
                om = rt.tile([128, 1], F32, tag=f"om{j}")
                nc.vector.tensor_scalar(om[:], s[:], -1.0, 1.0, ALU.mult, ALU.add)
                sel = rt.tile([128, E], F32, tag="selm")
                nc.vector.tensor_mul(sel[:], ca_a[:], sel_bc)
                cac = rt.tile([128, 1], F32, tag=f"cac{j}")
                nc.vector.tensor_reduce(
                    cac[:], sel[:], mybir.AxisListType.X, ALU.add
                )
                ca_cols.append(cac)
                oneminus.append(om)

            # ---- expert gate/up + SwiGLU -> h [DF_E, CT] -------------
            hc = []
            for f in range(KF):
                psg = ps_gu.tile([128, CT], F32, tag="psg")
                for k in range(KD):
                    nc.tensor.matmul(
                        psg[:],
                        gq[k][:, f * 128 : (f + 1) * 128],
                        xt[k][:],
                        start=(k == 0),
                        stop=(k == KD - 1),
                    )
                psu = ps_gu.tile([128, CT], F32, tag="psu")
                for k in range(KD):
                    nc.tensor.matmul(
                        psu[:],
                        uq[k][:, f * 128 : (f + 1) * 128],
                        xt[k][:],
                        start=(k == 0),
                        stop=(k == KD - 1),
                    )
                sig = work.tile([128, CT], F32, tag="sig")
                nc.scalar.activation(sig[:], psg[:], ACTF.Sigmoid)
                sil = work.tile([128, CT], F32, tag="sil")
                nc.vector.tensor_mul(sil[:], sig[:], psg[:])
                h = hp.tile([128, CT], DT_MM, tag=f"h{f}")
                nc.vector.tensor_mul(h[:], sil[:], psu[:])
                hc.append(h)

            # ---- shared gate/up -> hs [FS, CT] -----------------------
            hsc = []
            for f in range(KS):
                psg = ps_gu.tile([128, CT], F32, tag="psg")
                for k in range(KD):
                    nc.tensor.matmul(
                        psg[:],
                        wg[k][:, f * 128 : (f + 1) * 128],
                        xt[k][:],
                        start=(k == 0),
                        stop=(k == KD - 1),
                    )
                psu = ps_gu.tile([128, CT], F32, tag="psu")
                for k in range(KD):
                    nc.tensor.matmul(
                        psu[:],
                        wu[k][:, f * 128 : (f + 1) * 128],
                        xt[k][:],
                        start=(k == 0),
                        stop=(k == KD - 1),
                    )
                sig = work.tile([128, CT], F32, tag="sig")
                nc.scalar.activation(sig[:], psg[:], ACTF.Sigmoid)
                sil = work.tile([128, CT], F32, tag="sil")
                nc.vector.tensor_mul(sil[:], sig[:], psg[:])
                hs = hp.tile([128, CT], DT_MM, tag=f"hs{f}")
                nc.vector.tensor_mul(hs[:], sil[:], psu[:])
                hsc.append(hs)

            # ---- down projections + combine + scatter to bounce ------
            rs_in = dram.tile([CT, D], F32, tag=f"rsin{c}")
            rs_out = dram.tile([CT // NCORES, D], F32, tag=f"rsout{c}")
            for j in range(TT):
                for dd in range(ND):
                    pse = ps_dn.tile([128, 512], F32, tag="pse")
                    for k in range(KF):
                        nc.tensor.matmul(
                            pse[:],
                            hc[k][:, j * 128 : (j + 1) * 128],
                            dq[k][:, dd * 512 : (dd + 1) * 512],
                            start=(k == 0),
                            stop=(k == KF - 1),
                        )
                    pss = ps_sd.tile([128, 512], F32, tag="pss")
                    for k in range(KS):
                        nc.tensor.matmul(
                            pss[:],
                            hsc[k][:, j * 128 : (j + 1) * 128],
                            wd[k][:, dd * 512 : (dd + 1) * 512],
                            start=(k == 0),
                            stop=(k == KS - 1),
                        )
                    esc = work.tile([128, 512], F32, tag="esc")
                    nc.scalar.activation(
                        esc[:], pse[:], ACTF.Copy, scale=ca_cols[j][:]
                    )
                    contrib = work.tile([128, 512], F32, tag="contrib")
                    nc.vector.scalar_tensor_tensor(
                        contrib[:],
                        pss[:],
                        oneminus[j][:],
                        esc[:],
                        ALU.mult,
                        ALU.add,
                    )
                    nc.sync.dma_start(
                        rs_in[j * 128 : (j + 1) * 128, dd * 512 : (dd + 1) * 512],
                        contrib[:],
                    )

            nc.gpsimd.collective_compute(
                "ReduceScatter",
                ALU.add,
                replica_groups=[list(range(NCORES))],
                ins=[rs_in.opt()],
                outs=[rs_out.opt()],
            )
            # gpsimd queue: keeps this RS-dependent copy out of the sync
            # HWDGE FIFO so later chunks' input loads aren't blocked
            nc.gpsimd.dma_start(OUT[c], rs_out[:])

    nc.compile()
    return nc


def _prep_inputs(x, router_weight, sh_gate_w, sh_up_w, sh_down_w, gate_s,
                 up_s, down_s, alpha, gate_q, up_q, down_q):
    xf32 = np.ascontiguousarray(
        np.asarray(x, dtype=np.float32).reshape(T, D).T
    )
    xf = np.ascontiguousarray(xf32.astype(NP_MM))
    rwT = np.ascontiguousarray(np.asarray(router_weight, np.float32).T)
    in_maps = []
    for c in range(NCORES):
        gw = np.asarray(gate_q[c], np.float32) * np.asarray(
            gate_s[c], np.float32
        )[:, None]                                  # [DF_E, D]
        uw = np.asarray(up_q[c], np.float32) * np.asarray(
            up_s[c], np.float32
        )[:, None]                                  # [DF_E, D]
        dw = np.asarray(down_q[c], np.float32) * np.asarray(
            down_s[c], np.float32
        )[:, None]                                  # [D, DF_E]
        aux = np.zeros((128, 2 * E), np.float32)
        aux[:, 0:E] = np.asarray(alpha, np.float32)[None, :]
        aux[:, E + c] = 1.0
        in_maps.append(
            {
                "xT": xf,
                "xTf": xf32,
                "rwT": rwT,
                "gqT": np.ascontiguousarray(gw.T.astype(NP_MM)),
                "uqT": np.ascontiguousarray(uw.T.astype(NP_MM)),
                "dqT": np.ascontiguousarray(dw.T.astype(NP_MM)),
                "wgT": np.ascontiguousarray(
                    np.asarray(sh_gate_w[c * FS : (c + 1) * FS], np.float32)
                    .T.astype(NP_MM)
                ),
                "wuT": np.ascontiguousarray(
                    np.asarray(sh_up_w[c * FS : (c + 1) * FS], np.float32)
                    .T.astype(NP_MM)
                ),
                "wdT": np.ascontiguousarray(
                    np.asarray(sh_down_w[:, c * FS : (c + 1) * FS], np.float32)
                    .T.astype(NP_MM)
                ),
                "aux": aux,
            }
        )
    return in_maps


def kernel(x, router_weight, sh_gate_w, sh_up_w, sh_down_w, gate_s, up_s,
           down_s, alpha, gate_q, up_q, down_q, top_k, **run_kwargs):
    assert int(top_k) == 2, "kernel compiled for top_k=2"
    assert tuple(np.shape(x)) == (B, S, D)

    if "nc" not in _CACHE:
        _CACHE["nc"] = _build()
    nc = _CACHE["nc"]

    in_maps = _prep_inputs(
        x, router_weight, sh_gate_w, sh_up_w, sh_down_w, gate_s, up_s,
        down_s, alpha, gate_q, up_q, down_q,
    )
    res = run_bass_kernel_spmd(
        nc, in_maps, core_ids=list(range(NCORES)), **run_kwargs
    )
    _CACHE["last_results"] = res

    out = np.empty((T, D), np.float32)
    sh = CT // NCORES  # 64 rows per (chunk, rank)
    for r in range(NCORES):
        o = res.results[r]["OUT"]  # [CH, 64, D]
        for c in range(CH):
            out[c * CT + r * sh : c * CT + (r + 1) * sh] = o[c]
    return out.reshape(B, S, D).astype(np.asarray(x).dtype)


# revision 13
# speedup vs baseline: 1.2435x; 1.1837x over previous
"""Routed quantized MoE eval kernel for 8 Trainium2 NeuronCores.

Strategy (expert-parallel, per sharding hint):
- Core c owns expert e=c: quantized expert weights are dequantized
  (scale-folded) + transposed on the host at shard-prep time; the
  matmuls, router, top-2 softmax, SwiGLU activations and combine all
  run on device.
- Shared MLP is sharded along DF_S: core c computes rows
  [256c, 256c+256) of the shared gate/up and the matching columns of
  the down projection, giving a partial shared output.
- Every core computes the full router (fp32 matmuls - top-2 selection
  is tie-sensitive), forms its own expert's combine column
  ca[:, e] * alpha[e] and (1 - sum_e ca*alpha), scales its expert
  output and shared partial, and the per-token sum across all 8 cores
  is taken by chunked ReduceScatter collectives that overlap compute.
- Big matmuls run as float32r (2 cyc/row, ~1.5e-4 rel err), router in
  true float32.

Output identity used:
  mixed = (1 - sum_e ca_e*alpha_e) * shared + sum_e ca_e*alpha_e * eo_e
where shared = sum over cores of shared partials, so each core's
contribution is (1-s)*shared_partial_c + ca_c*alpha_c*eo_c.
"""

import numpy as np
from contextlib import ExitStack

import concourse.bass as bass
import concourse.tile as tile
from concourse import bacc, mybir
from concourse.bass_utils import run_bass_kernel_spmd

NCORES = 8
B, S, D = 2, 1024, 1024
T = B * S                      # 2048 tokens
DF_E, DF_S, E = 512, 2048, 8
FS = DF_S // NCORES            # 256 shared-ffn rows per core
CH = 4                         # token chunks
CT = T // CH                   # 512 tokens per chunk
TT = CT // 128                 # 4 token tiles per chunk
KD = D // 128                  # 8 k-tiles over hidden dim
KF = DF_E // 128               # 4 k-tiles over expert ffn dim
KS = FS // 128                 # 2 k-tiles over shared ffn shard
ND = D // 512                  # 2 output column slices

import os

FR = mybir.dt.float32r
F16 = mybir.dt.float16
F32 = mybir.dt.float32
DT_MODE = os.environ.get("MOE_DT", "f32r")
DT_MM = {"f32r": FR, "f16": F16}[DT_MODE]
NP_MM = {"f32r": np.float32, "f16": np.float16}[DT_MODE]
ACTF = mybir.ActivationFunctionType
ALU = mybir.AluOpType

_CACHE = {}


def _build():
    nc = bacc.Bacc(
        "TRN2", target_bir_lowering=False, debug=False, num_devices=NCORES
    )

    xT = nc.dram_tensor("xT", [D, T], DT_MM, kind="ExternalInput").ap()
    xTf = nc.dram_tensor("xTf", [D, T], F32, kind="ExternalInput").ap()
    rwT = nc.dram_tensor("rwT", [D, E], F32, kind="ExternalInput").ap()
    gqT = nc.dram_tensor("gqT", [D, DF_E], DT_MM, kind="ExternalInput").ap()
    uqT = nc.dram_tensor("uqT", [D, DF_E], DT_MM, kind="ExternalInput").ap()
    dqT = nc.dram_tensor("dqT", [DF_E, D], DT_MM, kind="ExternalInput").ap()
    wgT = nc.dram_tensor("wgT", [D, FS], DT_MM, kind="ExternalInput").ap()
    wuT = nc.dram_tensor("wuT", [D, FS], DT_MM, kind="ExternalInput").ap()
    wdT = nc.dram_tensor("wdT", [FS, D], DT_MM, kind="ExternalInput").ap()
    # aux[:, 0:8] = alpha broadcast, aux[:, 8:16] = onehot(expert) broadcast
    aux = nc.dram_tensor("aux", [128, 2 * E], F32, kind="ExternalInput").ap()
    OUT = nc.dram_tensor(
        "OUT", [CH, CT // NCORES, D], F32, kind="ExternalOutput"
    ).ap()

    with ExitStack() as ctx:
        tc = ctx.enter_context(tile.TileContext(nc))
        wres = ctx.enter_context(tc.tile_pool(name="wres", bufs=1))
        xs = ctx.enter_context(tc.tile_pool(name="xs", bufs=2))
        xfp = ctx.enter_context(tc.tile_pool(name="xfp", bufs=1))
        hp = ctx.enter_context(tc.tile_pool(name="hp", bufs=2))
        work = ctx.enter_context(tc.tile_pool(name="work", bufs=2))
        rt = ctx.enter_context(tc.tile_pool(name="rt", bufs=2))
        ps_gu = ctx.enter_context(tc.tile_pool(name="ps_gu", bufs=3, space="PSUM"))
        ps_dn = ctx.enter_context(tc.tile_pool(name="ps_dn", bufs=3, space="PSUM"))
        ps_r = ctx.enter_context(tc.tile_pool(name="ps_r", bufs=2, space="PSUM"))
        dram = ctx.enter_context(tc.tile_pool(name="dram", bufs=1, space="DRAM"))

        # ---- resident weights ----------------------------------------
        def load_rows(src, rows, cols, name):
            tiles = []
            r = src.rearrange("(k p) n -> k p n", p=128)
            for k in range(rows // 128):
                t = wres.tile([128, cols], src.dtype, tag=f"{name}{k}")
                nc.sync.dma_start(t[:], r[k])
                tiles.append(t)
            return tiles

        # router weights + aux first (tiny, unblock router matmuls)
        from concourse.masks import make_identity

        ident = wres.tile([128, 128], F32, tag="ident")
        make_identity(nc, ident[:])
        rw = load_rows(rwT, D, E, "rw")
        aux_sb = wres.tile([128, 2 * E], F32, tag="aux")
        nc.sync.dma_start(aux_sb[:], aux[:])
        alpha_bc = aux_sb[:, 0:E]
        sel_bc = aux_sb[:, E : 2 * E]

        xTr = xT.rearrange("(k p) t -> k p t", p=128)
        xTfr = xTf.rearrange("(k p) t -> k p t", p=128)

        def load_x(c):
            xt, xf_t = [], []
            for k in range(KD):
                tf = xfp.tile([128, CT], F32, tag=f"xf{k}")
                nc.sync.dma_start(tf[:], xTfr[k, :, c * CT : (c + 1) * CT])
                xf_t.append(tf)
                t = xs.tile([128, CT], DT_MM, tag=f"xt{k}")
                nc.sync.dma_start(t[:], xTr[k, :, c * CT : (c + 1) * CT])
                xt.append(t)
            return xt, xf_t

        x_pre = load_x(0)
        gq = load_rows(gqT, D, DF_E, "gq")
        uq = load_rows(uqT, D, DF_E, "uq")
        dq = load_rows(dqT, DF_E, D, "dq")
        wg = load_rows(wgT, D, FS, "wg")
        wu = load_rows(wuT, D, FS, "wu")
        wd = load_rows(wdT, FS, D, "wd")

        for c in range(CH):
            xt, xf_t = x_pre
            if c + 1 < CH:
                x_pre = load_x(c + 1)

            # ---- router + combine weights ----------------------------
            ca_cols = []
            oneminus = []
            ps_lt = ps_r.tile([E, CT], F32, tag="psr")
            for k in range(KD):
                nc.tensor.matmul(
                    ps_lt[:],
                    rw[k][:],
                    xf_t[k][:],
                    start=(k == 0),
                    stop=(k == KD - 1),
                )
            Lt = rt.tile([E, CT], F32, tag="Lt")
            nc.vector.tensor_copy(Lt[:], ps_lt[:])
            for j in range(TT):
                ps_l = ps_r.tile([128, E], F32, tag="psr")
                nc.tensor.transpose(
                    ps_l[:], Lt[:, j * 128 : (j + 1) * 128], ident[0:E, 0:E]
                )
                L = rt.tile([128, E], F32, tag="L")
                nc.vector.tensor_copy(L[:], ps_l[:])
                m1 = rt.tile([128, 1], F32, tag="m1")
                nc.vector.tensor_reduce(m1[:], L[:], mybir.AxisListType.X, ALU.max)
                mask1 = rt.tile([128, E], F32, tag="mask1")
                nc.vector.tensor_scalar(mask1[:], L[:], m1[:], None, ALU.is_ge)
                L2 = rt.tile([128, E], F32, tag="L2")
                nc.vector.scalar_tensor_tensor(
                    L2[:], mask1[:], -1e30, L[:], ALU.mult, ALU.add
                )
                m2 = rt.tile([128, 1], F32, tag="m2")
                nc.vector.tensor_reduce(m2[:], L2[:], mybir.AxisListType.X, ALU.max)
                mask2 = rt.tile([128, E], F32, tag="mask2")
                nc.vector.tensor_scalar(mask2[:], L2[:], m2[:], None, ALU.is_ge)
                # softmax over {m1, m2}: w1 = sigmoid(m1 - m2), w2 = 1 - w1
                dlt = rt.tile([128, 1], F32, tag="dlt")
                nc.vector.tensor_sub(dlt[:], m1[:], m2[:])
                w1 = rt.tile([128, 1], F32, tag="w1")
                nc.scalar.activation(w1[:], dlt[:], ACTF.Sigmoid)
                w2 = rt.tile([128, 1], F32, tag="w2")
                nc.vector.tensor_scalar(w2[:], w1[:], -1.0, 1.0, ALU.mult, ALU.add)
                caw = rt.tile([128, E], F32, tag="caw")
                nc.vector.tensor_scalar(caw[:], mask2[:], w2[:], None, ALU.mult)
                nc.vector.scalar_tensor_tensor(
                    caw[:], mask1[:], w1[:], caw[:], ALU.mult, ALU.add
                )
                ca_a = rt.tile([128, E], F32, tag="ca_a")
                nc.vector.tensor_mul(ca_a[:], caw[:], alpha_bc)
                s = rt.tile([128, 1], F32, tag="s")
                nc.vector.tensor_reduce(s[:], ca_a[:], mybir.AxisListType.X, ALU.add)
                om = rt.tile([128, 1], F32, tag=f"om{j}")
                nc.vector.tensor_scalar(om[:], s[:], -1.0, 1.0, ALU.mult, ALU.add)
                sel = rt.tile([128, E], F32, tag="selm")
                nc.vector.tensor_mul(sel[:], ca_a[:], sel_bc)
                cac = rt.tile([128, 1], F32, tag=f"cac{j}")
                nc.vector.tensor_reduce(
                    cac[:], sel[:], mybir.AxisListType.X, ALU.add
                )
                ca_cols.append(cac)
                oneminus.append(om)

            # ---- expert gate/up + SwiGLU -> h [DF_E, CT] -------------
            hc = []
            for f in range(KF):
                psg = ps_gu.tile([128, CT], F32, tag="psgu")
                for k in range(KD):
                    nc.tensor.matmul(
                        psg[:],
                        gq[k][:, f * 128 : (f + 1) * 128],
                        xt[k][:],
                        start=(k == 0),
                        stop=(k == KD - 1),
                    )
                psu = ps_gu.tile([128, CT], F32, tag="psgu")
                for k in range(KD):
                    nc.tensor.matmul(
                        psu[:],
                        uq[k][:, f * 128 : (f + 1) * 128],
                        xt[k][:],
                        start=(k == 0),
                        stop=(k == KD - 1),
                    )
                sig = work.tile([128, CT], F32, tag="sig")
                nc.scalar.activation(sig[:], psg[:], ACTF.Sigmoid)
                sil = work.tile([128, CT], F32, tag="sil")
                nc.vector.tensor_mul(sil[:], sig[:], psg[:])
                h = hp.tile([128, CT], DT_MM, tag=f"h{f}")
                nc.vector.tensor_mul(h[:], sil[:], psu[:])
                hc.append(h)

            # ---- shared gate/up -> hs [FS, CT] -----------------------
            hsc = []
            for f in range(KS):
                psg = ps_gu.tile([128, CT], F32, tag="psgu")
                for k in range(KD):
                    nc.tensor.matmul(
                        psg[:],
                        wg[k][:, f * 128 : (f + 1) * 128],
                        xt[k][:],
                        start=(k == 0),
                        stop=(k == KD - 1),
                    )
                psu = ps_gu.tile([128, CT], F32, tag="psgu")
                for k in range(KD):
                    nc.tensor.matmul(
                        psu[:],
                        wu[k][:, f * 128 : (f + 1) * 128],
                        xt[k][:],
                        start=(k == 0),
                        stop=(k == KD - 1),
                    )
                sig = work.tile([128, CT], F32, tag="sig")
                nc.scalar.activation(sig[:], psg[:], ACTF.Sigmoid)
                sil = work.tile([128, CT], F32, tag="sil")
                nc.vector.tensor_mul(sil[:], sig[:], psg[:])
                hs = hp.tile([128, CT], DT_MM, tag=f"hs{f}")
                nc.vector.tensor_mul(hs[:], sil[:], psu[:])
                hsc.append(hs)

            # ---- down projections + combine + scatter to bounce ------
            rs_in = dram.tile([CT, D], F32, tag=f"rsin{c}")
            rs_out = dram.tile([CT // NCORES, D], F32, tag=f"rsout{c}")
            for j in range(TT):
                for dd in range(ND):
                    pse = ps_dn.tile([128, 512], F32, tag="psd")
                    for k in range(KF):
                        nc.tensor.matmul(
                            pse[:],
                            hc[k][:, j * 128 : (j + 1) * 128],
                            dq[k][:, dd * 512 : (dd + 1) * 512],
                            start=(k == 0),
                            stop=(k == KF - 1),
                        )
                    pss = ps_dn.tile([128, 512], F32, tag="psd")
                    for k in range(KS):
                        nc.tensor.matmul(
                            pss[:],
                            hsc[k][:, j * 128 : (j + 1) * 128],
                            wd[k][:, dd * 512 : (dd + 1) * 512],
                            start=(k == 0),
                            stop=(k == KS - 1),
                        )
                    esc = work.tile([128, 512], F32, tag="esc")
                    nc.vector.tensor_scalar(
                        esc[:], pse[:], ca_cols[j][:], None, ALU.mult
                    )
                    contrib = work.tile([128, 512], F32, tag="contrib")
                    nc.vector.scalar_tensor_tensor(
                        contrib[:],
                        pss[:],
                        oneminus[j][:],
                        esc[:],
                        ALU.mult,
                        ALU.add,
                    )
                    nc.sync.dma_start(
                        rs_in[j * 128 : (j + 1) * 128, dd * 512 : (dd + 1) * 512],
                        contrib[:],
                    )

            nc.gpsimd.collective_compute(
                "ReduceScatter",
                ALU.add,
                replica_groups=[list(range(NCORES))],
                ins=[rs_in.opt()],
                outs=[rs_out.opt()],
            )
            # gpsimd queue: keeps this RS-dependent copy out of the sync
            # HWDGE FIFO so later chunks' input loads aren't blocked
            nc.gpsimd.dma_start(OUT[c], rs_out[:])

    nc.compile()
    return nc


def _prep_inputs(x, router_weight, sh_gate_w, sh_up_w, sh_down_w, gate_s,
                 up_s, down_s, alpha, gate_q, up_q, down_q):
    xf32 = np.ascontiguousarray(
        np.asarray(x, dtype=np.float32).reshape(T, D).T
    )
    xf = np.ascontiguousarray(xf32.astype(NP_MM))
    rwT = np.ascontiguousarray(np.asarray(router_weight, np.float32).T)
    in_maps = []
    for c in range(NCORES):
        gw = np.asarray(gate_q[c], np.float32) * np.asarray(
            gate_s[c], np.float32
        )[:, None]                                  # [DF_E, D]
        uw = np.asarray(up_q[c], np.float32) * np.asarray(
            up_s[c], np.float32
        )[:, None]                                  # [DF_E, D]
        dw = np.asarray(down_q[c], np.float32) * np.asarray(
            down_s[c], np.float32
        )[:, None]                                  # [D, DF_E]
        aux = np.zeros((128, 2 * E), np.float32)
        aux[:, 0:E] = np.asarray(alpha, np.float32)[None, :]
        aux[:, E + c] = 1.0
        in_maps.append(
            {
                "xT": xf,
                "xTf": xf32,
                "rwT": rwT,
                "gqT": np.ascontiguousarray(gw.T.astype(NP_MM)),
                "uqT": np.ascontiguousarray(uw.T.astype(NP_MM)),
                "dqT": np.ascontiguousarray(dw.T.astype(NP_MM)),
                "wgT": np.ascontiguousarray(
                    np.asarray(sh_gate_w[c * FS : (c + 1) * FS], np.float32)
                    .T.astype(NP_MM)
                ),
                "wuT": np.ascontiguousarray(
                    np.asarray(sh_up_w[c * FS : (c + 1) * FS], np.float32)
                    .T.astype(NP_MM)
                ),
                "wdT": np.ascontiguousarray(
                    np.asarray(sh_down_w[:, c * FS : (c + 1) * FS], np.float32)
                    .T.astype(NP_MM)
                ),
                "aux": aux,
            }
        )
    return in_maps


def kernel(x, router_weight, sh_gate_w, sh_up_w, sh_down_w, gate_s, up_s,
           down_s, alpha, gate_q, up_q, down_q, top_k, **run_kwargs):
    assert int(top_k) == 2, "kernel compiled for top_k=2"
    assert tuple(np.shape(x)) == (B, S, D)

    if "nc" not in _CACHE:
        _CACHE["nc"] = _build()
    nc = _CACHE["nc"]

    in_maps = _prep_inputs(
        x, router_weight, sh_gate_w, sh_up_w, sh_down_w, gate_s, up_s,
        down_s, alpha, gate_q, up_q, down_q,
    )
    res = run_bass_kernel_spmd(
        nc, in_maps, core_ids=list(range(NCORES)), **run_kwargs
    )
    _CACHE["last_results"] = res

    out = np.empty((T, D), np.float32)
    sh = CT // NCORES  # 64 rows per (chunk, rank)
    for r in range(NCORES):
        o = res.results[r]["OUT"]  # [CH, 64, D]
        for c in range(CH):
            out[c * CT + r * sh : c * CT + (r + 1) * sh] = o[c]
    return out.reshape(B, S, D).astype(np.asarray(x).dtype)
